# revision 7
# baseline (speedup 1.0000x reference)
"""Trainium2 Bass kernel for nn_DecoderTreeNN (gather + segment_sum over trees).

Computes, for two embedding tables C_hop / C_hop1:
    out[t, seg, :] = sum_{i : tree_ids[i] == seg} C_t[token_ids[i], :]
returning [2, 32, 512, 128] f32.

Strategy (8 NeuronCores, SPMD):
  Algebraic regrouping: out[seg, :] = sum_v H[seg, v] * C[v, :], where
  H[seg, v] = multiplicity of vocab id v among the tokens of segment seg.
  H is pure index bookkeeping (a histogram over (tree_id, token_id) pairs),
  computed on host exactly like the baseline's window packing; every float
  multiply/add runs on device as a dense GEMM.

  - Segments are sharded across cores: core c owns segs [2048c, 2048(c+1)).
  - Device computes outT[d, s] = sum_v C[v, d] * H^T[v, s] per core:
    a [256, 32000] x [32000, 2048] GEMM = 33.5 GFLOP bf16 (~430 us at peak).
  - The concatenated table C [32000, 256] lives bf16-resident in SBUF
    (125 KB/partition), laid out [128 v_lo, 250 k * 256 d] so each k-tile
    slice is a natural [128, 128] lhsT (stationary operand, FWL-eligible).
  - H^T bf16 (131 MB/core, counts <= 255 are exact in bf16) streams from
    HBM in 2.6 MB coalesced DMAs (5 k-tiles per transfer), triple-buffered.
  - PSUM holds the entire per-core output: 2 d-halves x 4 s-chunks of
    [128, 512] f32 = all 8 banks, accumulated across the 250 k-tiles
    (start at k=0, stop at k=249), then copied out via DVE and DMA'd.
  - No collectives: per-core outputs are disjoint; host restacks.
"""

from contextlib import ExitStack

import numpy as np
import ml_dtypes

import concourse.bacc as bacc
import concourse.bass as bass
import concourse.mybir as mybir
import concourse.tile as tile
from concourse.bass_utils import run_bass_kernel_spmd

P = 128
V = 32000
D = 128              # embedding dim per table
DD = 2 * D           # concatenated row width
N_CORES = 8
NSEG = 16384
SEG_C = NSEG // N_CORES          # 2048 segments per core
KT = V // P                      # 250 contraction tiles
GROUP = 10                       # k-tiles per H DMA (2.62 MB fp8 transfers)
NG = KT // GROUP                 # 25 DMAs per rep
SW = 512                         # matmul moving free dim / PSUM bank (f32)
SC = SEG_C // SW                 # 4 s-chunks

_compiled = None


def _build_program(reps=1):
    nc = bacc.Bacc(
        "TRN2", target_bir_lowering=False, debug=False, num_devices=N_CORES
    )
    t_cb = nc.dram_tensor("cb", [P, KT * DD], mybir.dt.bfloat16, kind="ExternalInput")
    t_ht = nc.dram_tensor(
        "ht", [NG, P, GROUP * SEG_C], mybir.dt.float8e4, kind="ExternalInput"
    )
    t_out = nc.dram_tensor(
        "out", [reps * 2, P, SEG_C], mybir.dt.float32, kind="ExternalOutput"
    )

    with tile.TileContext(nc) as tc, ExitStack() as ctx:
        const = ctx.enter_context(tc.tile_pool(name="const", bufs=1))
        hpool = ctx.enter_context(tc.tile_pool(name="h", bufs=3))
        opool = ctx.enter_context(tc.tile_pool(name="o", bufs=2))
        ppool = ctx.enter_context(tc.tile_pool(name="p", bufs=1, space="PSUM"))

        cb = const.tile([P, KT * DD], mybir.dt.bfloat16)
        nc.sync.dma_start(cb[:], t_cb[:])

        for r in range(reps):
            ps = [
                ppool.tile([P, SW], mybir.dt.float32, tag=f"ps{j}", name=f"ps{j}")
                for j in range(8)
            ]
            for g in range(NG):
                hb = hpool.tile([P, GROUP * SEG_C], mybir.dt.float8e4, tag="h")
                nc.sync.dma_start(hb[:], t_ht[g])
                for i in range(GROUP):
                    k = GROUP * g + i
                    for h in range(2):
                        lw = cb[:, k * DD + D * h : k * DD + D * h + D]
                        for sc in range(SC):
                            nc.tensor.matmul(
                                out=ps[4 * h + sc][:],
                                lhsT=lw,
                                rhs=hb[:, i * SEG_C + SW * sc : i * SEG_C + SW * (sc + 1)],
                                start=(k == 0),
                                stop=(k == KT - 1),
                            )
            for h in range(2):
                for sc in range(SC):
                    ot = opool.tile([P, SW], mybir.dt.float32, tag="o")
                    nc.vector.tensor_copy(out=ot[:], in_=ps[4 * h + sc][:])
                    nc.sync.dma_start(
                        t_out[2 * r + h][:, SW * sc : SW * (sc + 1)], ot[:]
                    )

    nc.compile()
    return nc


def _pack_inputs(token_ids, tree_ids, C_hop, C_hop1):
    """Host-side index bookkeeping + layout; no float arithmetic on the data.

    Returns
      cb: [128, 250*256] bf16 — table, cb[p, k*256+d] = C2[128k+p, d]
      ht: [8, 50, 128, 5*2048] bf16 — per-core H^T tiles,
          ht[c, g, p, i*2048+s] = H[2048c+s, 128*(5g+i)+p]
    """
    tok = np.asarray(token_ids).astype(np.int64)
    tree = np.asarray(tree_ids).astype(np.int64)

    C2 = np.concatenate(
        [np.asarray(C_hop, np.float32), np.asarray(C_hop1, np.float32)], axis=1
    ).astype(ml_dtypes.bfloat16)
    cb = np.ascontiguousarray(C2.reshape(KT, P, DD).transpose(1, 0, 2).reshape(P, KT * DD))

    bounds = np.searchsorted(tree, np.arange(0, NSEG + 1, SEG_C))
    # e4m3 represents integers 0..16 exactly; counts here are tiny (max ~3)
    lut = np.arange(17, dtype=np.float32).astype(ml_dtypes.float8_e4m3).view(np.uint8)
    ht = np.empty((N_CORES, NG, P, GROUP * SEG_C), np.uint8)
    for c in range(N_CORES):
        s, e = bounds[c], bounds[c + 1]
        flat = (tree[s:e] - c * SEG_C) * V + tok[s:e]
        cnt = np.bincount(flat, minlength=SEG_C * V)
        assert cnt.max() <= 16, f"count {cnt.max()} not exact in e4m3"
        u = lut[cnt].reshape(SEG_C, V)
        ht[c] = (
            u.T.reshape(NG, GROUP, P, SEG_C)
            .transpose(0, 2, 1, 3)
            .reshape(NG, P, GROUP * SEG_C)
        )
    return cb, ht.view(ml_dtypes.float8_e4m3)


def kernel(token_ids, tree_ids, C_hop, C_hop1, batch_size, max_trees):
    global _compiled
    batch_size = int(batch_size)
    max_trees = int(max_trees)
    assert batch_size * max_trees == NSEG

    cb, ht = _pack_inputs(token_ids, tree_ids, C_hop, C_hop1)

    if _compiled is None:
        _compiled = _build_program()
    nc = _compiled

    in_maps = [{"cb": cb, "ht": ht[c]} for c in range(N_CORES)]
    res = run_bass_kernel_spmd(nc, in_maps, core_ids=list(range(N_CORES)))

    # res[c]["out"] = [2, 128, 2048]: outT[d-half, d_lo, s] for segs 2048c+s
    allseg = np.concatenate(
        [
            np.concatenate(
                [res.results[c]["out"][0], res.results[c]["out"][1]], axis=0
            ).T
            for c in range(N_CORES)
        ],
        axis=0,
    )  # [16384, 256]
    key = allseg[:, :D].reshape(batch_size, max_trees, D)
    val = allseg[:, D:].reshape(batch_size, max_trees, D)
    return np.stack([key, val]).astype(np.float32)


# revision 11
# speedup vs baseline: 1.0971x; 1.0971x over previous
"""Trainium2 Bass kernel for nn_DecoderTreeNN (gather + segment_sum over trees).

Computes, for two embedding tables C_hop / C_hop1:
    out[t, seg, :] = sum_{i : tree_ids[i] == seg} C_t[token_ids[i], :]
returning [2, 32, 512, 128] f32.

Strategy (8 NeuronCores, SPMD):
  Algebraic regrouping: out[seg, :] = sum_v H[seg, v] * C[v, :], where
  H[seg, v] = multiplicity of vocab id v among the tokens of segment seg.
  H is pure index bookkeeping (a histogram over (tree_id, token_id) pairs),
  computed on host exactly like the baseline's window packing; every float
  multiply/add runs on device as a dense GEMM.

  - Segments are sharded across cores: core c owns segs [2048c, 2048(c+1)).
  - Device computes outT[d, s] = sum_v C[v, d] * H^T[v, s] per core:
    a [256, 32000] x [32000, 2048] GEMM, 33.5 GFLOP per core.
  - H^T is fp8-e4m3 (counts here are tiny ints <= 16, exact in e4m3);
    65 MB/core streams from HBM in 2.6 MB coalesced DMAs, triple-buffered.
  - The table C is split C = C_hi + C_lo with C_hi = e4m3(C) and
    C_lo = e4m3(C - C_hi) (~10-bit effective mantissa, better than the
    bf16 gate needs). The fp8 DoubleRow perf mode contracts the (hi, lo)
    pair in ONE matmul at 0.5 cycles/row — 2x the bf16 PE rate — with the
    rhs pair dimension a stride-0 broadcast of the same H tile, so H is
    not duplicated. PSUM accumulates both pair products in f32.
  - The pair-split table lives resident in SBUF ([128, 250*2*256] fp8 =
    125 KB/partition); PSUM holds the whole per-core output (2 d-halves x
    4 s-chunks of [128, 512] f32 = all 8 banks) accumulated across the
    250 k-tiles, then drains via DVE -> SBUF -> DMA.
  - No collectives: per-core outputs are disjoint; host restacks.
"""

from contextlib import ExitStack

import numpy as np
import ml_dtypes

import concourse.bacc as bacc
import concourse.bass as bass
import concourse.mybir as mybir
import concourse.tile as tile
from concourse.bass_utils import run_bass_kernel_spmd

P = 128
V = 32000
D = 128              # embedding dim per table
DD = 2 * D           # concatenated row width
N_CORES = 8
NSEG = 16384
SEG_C = NSEG // N_CORES          # 2048 segments per core
KT = V // P                      # 250 contraction tiles
GROUP = 10                       # k-tiles per H DMA (2.62 MB fp8 transfers)
NG = KT // GROUP                 # 25 DMAs per rep
SW = 512                         # matmul moving free dim / PSUM bank (f32)
SC = SEG_C // SW                 # 4 s-chunks

_compiled = None


def _build_program(reps=1):
    nc = bacc.Bacc(
        "TRN2", target_bir_lowering=False, debug=False, num_devices=N_CORES
    )
    t_cb = nc.dram_tensor(
        "cb", [P, KT, 2, DD], mybir.dt.float8e4, kind="ExternalInput"
    )
    t_ht = nc.dram_tensor(
        "ht", [NG, P, GROUP * SEG_C], mybir.dt.float8e4, kind="ExternalInput"
    )
    t_out = nc.dram_tensor(
        "out", [reps * 2, P, SEG_C], mybir.dt.float32, kind="ExternalOutput"
    )

    with tile.TileContext(nc) as tc, ExitStack() as ctx:
        const = ctx.enter_context(tc.tile_pool(name="const", bufs=1))
        hpool = ctx.enter_context(tc.tile_pool(name="h", bufs=3))
        opool = ctx.enter_context(tc.tile_pool(name="o", bufs=2))
        ppool = ctx.enter_context(tc.tile_pool(name="p", bufs=1, space="PSUM"))

        cbt = const.tile([P, KT, 2, DD], mybir.dt.float8e4, name="cbt")
        nc.sync.dma_start(cbt[:], t_cb[:])

        for r in range(reps):
            ps = [
                ppool.tile([P, SW], mybir.dt.float32, tag=f"ps{j}", name=f"ps{j}")
                for j in range(8)
            ]
            for g in range(NG):
                hb = hpool.tile([P, GROUP * SEG_C], mybir.dt.float8e4, tag="h", name="hb")
                nc.sync.dma_start(hb[:], t_ht[g])
                for i in range(GROUP):
                    k = GROUP * g + i
                    for h in range(2):
                        lw = cbt[:, k, :, D * h : D * (h + 1)]
                        for sc in range(SC):
                            rhs = (
                                hb[:, i * SEG_C + SW * sc : i * SEG_C + SW * (sc + 1)]
                                .unsqueeze(1)
                                .broadcast_to([P, 2, SW])
                            )
                            nc.tensor.matmul(
                                out=ps[4 * h + sc][:],
                                lhsT=lw,
                                rhs=rhs,
                                start=(k == 0),
                                stop=(k == KT - 1),
                                perf_mode=mybir.MatmulPerfMode.DoubleRow,
                            )
            for h in range(2):
                for sc in range(SC):
                    ot = opool.tile([P, SW], mybir.dt.float32, tag="o", name="ot")
                    nc.vector.tensor_copy(out=ot[:], in_=ps[4 * h + sc][:])
                    nc.sync.dma_start(
                        t_out[2 * r + h][:, SW * sc : SW * (sc + 1)], ot[:]
                    )

    nc.compile()
    return nc


def _pack_inputs(token_ids, tree_ids, C_hop, C_hop1):
    """Host-side index bookkeeping + layout; no float arithmetic on the data.

    Returns
      cb: [128, 250*2*256] fp8 — pair-split table,
          cb[p, (k*2+j)*256+d] = (C_hi if j==0 else C_lo)[128k+p, d]
      ht: [8, 25, 128, 10*2048] fp8 — per-core H^T tiles,
          ht[c, g, p, i*2048+s] = H[2048c+s, 128*(10g+i)+p]
    """
    tok = np.asarray(token_ids).astype(np.int64)
    tree = np.asarray(tree_ids).astype(np.int64)

    C2 = np.concatenate(
        [np.asarray(C_hop, np.float32), np.asarray(C_hop1, np.float32)], axis=1
    )
    c_hi = C2.astype(ml_dtypes.float8_e4m3)
    c_lo = (C2 - c_hi.astype(np.float32)).astype(ml_dtypes.float8_e4m3)
    # [32000, 2, 256] pair-interleaved -> [128, 250, 2, 256] -> flat
    cpair = np.stack([c_hi, c_lo], axis=1)
    cb = np.ascontiguousarray(cpair.reshape(KT, P, 2, DD).transpose(1, 0, 2, 3))

    bounds = np.searchsorted(tree, np.arange(0, NSEG + 1, SEG_C))
    # e4m3 represents integers 0..16 exactly; counts here are tiny (max ~3)
    lut = np.arange(17, dtype=np.float32).astype(ml_dtypes.float8_e4m3).view(np.uint8)
    ht = np.empty((N_CORES, NG, P, GROUP * SEG_C), np.uint8)
    for c in range(N_CORES):
        s, e = bounds[c], bounds[c + 1]
        flat = (tree[s:e] - c * SEG_C) * V + tok[s:e]
        cnt = np.bincount(flat, minlength=SEG_C * V)
        assert cnt.max() <= 16, f"count {cnt.max()} not exact in e4m3"
        u = lut[cnt].reshape(SEG_C, V)
        ht[c] = (
            u.T.reshape(NG, GROUP, P, SEG_C)
            .transpose(0, 2, 1, 3)
            .reshape(NG, P, GROUP * SEG_C)
        )
    return cb, ht.view(ml_dtypes.float8_e4m3)


def kernel(token_ids, tree_ids, C_hop, C_hop1, batch_size, max_trees):
    global _compiled
    batch_size = int(batch_size)
    max_trees = int(max_trees)
    assert batch_size * max_trees == NSEG

    cb, ht = _pack_inputs(token_ids, tree_ids, C_hop, C_hop1)

    if _compiled is None:
        _compiled = _build_program()
    nc = _compiled

    in_maps = [{"cb": cb, "ht": ht[c]} for c in range(N_CORES)]
    res = run_bass_kernel_spmd(nc, in_maps, core_ids=list(range(N_CORES)))

    # res[c]["out"] = [2, 128, 2048]: outT[d-half, d_lo, s] for segs 2048c+s
    allseg = np.concatenate(
        [
            np.concatenate(
                [res.results[c]["out"][0], res.results[c]["out"][1]], axis=0
            ).T
            for c in range(N_CORES)
        ],
        axis=0,
    )  # [16384, 256]
    key = allseg[:, :D].reshape(batch_size, max_trees, D)
    val = allseg[:, D:].reshape(batch_size, max_trees, D)
    return np.stack([key, val]).astype(np.float32)
